# revision 3
# baseline (speedup 1.0000x reference)
"""Trainium2 Bass kernel for nn_ATT_ONE2MANY_1.

reference:
    proj = tanh(many @ W)                 # [B,T,D]
    logits = einsum('btd,bd->bt', proj, one)
    att = softmax(logits) with EPS=1e-7 in denominator
    result = einsum('btd,bt->bd', many, att)
    returns (result, att)

Sharding: data-parallel over batch B=32 across 8 cores (4 samples/core),
W replicated. No cross-core comms.

Per-core dataflow (all matmul data in float32r — ~13 mantissa bits, full PE rate):
  for s in 0..3:
    for h in 0..1 (T-halves):
      transpose many[s, h] tiles on PE -> manyT[e][128, 1024]  (8 e-chunks)
      for d0 in 0..7:
        projT psum[128,512]x2 accumulated over e (W[e,d0] stationary)
        tanh (ACT, PSUM->SBUF, f32r)
        logits[1,512]x2 psum += oneT[:,d0,s].T @ tanhproj  (accum over d0)
    softmax on [1, 2048] row (exact max, exp on ACT with accum sum, EPS=1e-7)
    attT via 16 PE micro-transposes
    wsum: result[1,1024] psum += attT[:,j].T @ many[s,j] (re-streamed from DRAM)
"""

import numpy as np

B, T, D = 32, 2048, 1024
NCORES = 8
BLOC = B // NCORES  # 4
EPS = 1e-7

_CACHE = {}


def _build(reps: int = 1):
    """Build the Bass program (one core's SPMD program). reps>1 wraps the
    whole body in a hardware loop for wall-clock timing."""
    from contextlib import ExitStack
    import concourse.bass as bass
    import concourse.tile as tile
    import concourse.mybir as mybir
    from concourse import bacc
    from concourse.masks import make_identity

    f32 = mybir.dt.float32
    f32r = mybir.dt.float32r
    Tanh = mybir.ActivationFunctionType.Tanh
    Exp = mybir.ActivationFunctionType.Exp

    nc = bacc.Bacc("TRN2", debug=False, num_devices=NCORES)

    one_d = nc.dram_tensor("one", [BLOC, D], f32r, kind="ExternalInput").ap()
    many_d = nc.dram_tensor("many", [BLOC, T, D], f32r, kind="ExternalInput").ap()
    w_d = nc.dram_tensor("W", [D, D], f32r, kind="ExternalInput").ap()
    res_d = nc.dram_tensor("result", [BLOC, D], f32, kind="ExternalOutput").ap()
    att_d = nc.dram_tensor("att", [BLOC, T], f32, kind="ExternalOutput").ap()

    ED = D // 128   # 8 e-chunks (contraction dim of mm1)
    ND = D // 128   # 8 d0-chunks (output dim of mm1)
    TC = T // 128   # 16 t-chunks per sample
    HC = TC // 2    # 8 t-chunks per half

    with tile.TileContext(nc) as tc, ExitStack() as ctx:
        # ---- pools ----
        consts = ctx.enter_context(tc.tile_pool(name="consts", bufs=1))
        natp = ctx.enter_context(tc.tile_pool(name="natp", bufs=3))
        mtp = ctx.enter_context(tc.tile_pool(name="mtp", bufs=2))
        thp = ctx.enter_context(tc.tile_pool(name="thp", bufs=2))
        smallp = ctx.enter_context(tc.tile_pool(name="smallp", bufs=2))
        # PSUM: tr(2) + pj0(2) + pj1(2) + lgres(2) = 8 banks
        ps_tr = ctx.enter_context(tc.tile_pool(name="ps_tr", bufs=2, space="PSUM"))
        ps_pj = ctx.enter_context(tc.tile_pool(name="ps_pj", bufs=2, space="PSUM"))
        ps_lg = ctx.enter_context(tc.tile_pool(name="ps_lg", bufs=1, space="PSUM"))

        # ---- constants / setup ----
        ident_f = consts.tile([128, 128], f32, tag="identf")
        make_identity(nc, ident_f[:])
        ident = consts.tile([128, 128], f32r, tag="ident")
        nc.vector.tensor_copy(ident[:], ident_f[:])

        w_sb = []
        for e in range(ED):
            wt = consts.tile([128, D], f32r, name=f"w{e}", tag=f"w{e}")
            nc.sync.dma_start(wt[:], w_d[128 * e:128 * (e + 1), :])
            w_sb.append(wt)

        one_sb = consts.tile([BLOC, D], f32r, tag="one")
        nc.sync.dma_start(one_sb[:], one_d[:])
        # oneT[:, d0*BLOC + s] = one[s, 128*d0 + :]
        oneT = consts.tile([128, ND * BLOC], f32r, tag="oneT")
        for d0 in range(ND):
            tp = ps_tr.tile([128, 128], f32r, name=f"onetp{d0}", tag="tr")
            nc.tensor.matmul(tp[0:128, 0:BLOC], one_sb[0:BLOC, 128 * d0:128 * (d0 + 1)],
                             ident[0:BLOC, 0:BLOC], is_transpose=True)
            nc.vector.tensor_copy(oneT[:, BLOC * d0:BLOC * (d0 + 1)], tp[0:128, 0:BLOC])

        def body(rep):
            for s in range(BLOC):
                logits_sb = smallp.tile([1, T], f32, name=f"lgs{rep}_{s}", tag="logits")
                for h in range(2):
                    # ---- transpose phase: manyT[e] [128(e), 1024(t of half)] ----
                    manyT = [mtp.tile([128, 8 * 128], f32r, name=f"mT{rep}_{s}_{h}_{e}", tag=f"mT{e}")
                             for e in range(ED)]
                    for c in range(HC):
                        t0 = 1024 * h + 128 * c
                        nat = natp.tile([128, D], f32r, name=f"nat{rep}_{s}_{h}_{c}", tag="nat")
                        nc.sync.dma_start(nat[:], many_d[s, t0:t0 + 128, :])
                        for e in range(ED):
                            tp = ps_tr.tile([128, 128], f32r, name=f"tp{rep}_{s}_{h}_{c}_{e}", tag="tr")
                            nc.tensor.matmul(tp[:], nat[:, 128 * e:128 * (e + 1)], ident[:],
                                             is_transpose=True)
                            nc.vector.tensor_copy(manyT[e][:, 128 * c:128 * (c + 1)], tp[:])
                    # ---- mm1 + tanh + mm2 ----
                    lg = ps_lg.tile([1, 1024], f32, name=f"lg{rep}_{s}_{h}", tag="lgres")
                    for d0 in range(ND):
                        pj = [ps_pj.tile([128, 512], f32, name=f"pj{rep}_{s}_{h}_{d0}_{tb}", tag=f"pj{tb}")
                              for tb in range(2)]
                        for e in range(ED):
                            for tb in range(2):
                                nc.tensor.matmul(
                                    pj[tb][:],
                                    w_sb[e][:, 128 * d0:128 * (d0 + 1)],
                                    manyT[e][:, 512 * tb:512 * (tb + 1)],
                                    start=(e == 0), stop=(e == ED - 1))
                        th = thp.tile([128, 1024], f32r, name=f"th{rep}_{s}_{h}_{d0}", tag="th")
                        for tb in range(2):
                            nc.scalar.activation(th[:, 512 * tb:512 * (tb + 1)], pj[tb][:], Tanh)
                        ocol = oneT[:, BLOC * d0 + s:BLOC * d0 + s + 1]
                        for tb in range(2):
                            nc.tensor.matmul(lg[0:1, 512 * tb:512 * (tb + 1)],
                                             ocol, th[:, 512 * tb:512 * (tb + 1)],
                                             start=(d0 == 0), stop=(d0 == ND - 1))
                    nc.vector.tensor_copy(logits_sb[0:1, 1024 * h:1024 * (h + 1)], lg[:])

                # ---- softmax on [1, T] (partition 0) ----
                negmax = smallp.tile([1, 1], f32, name=f"nm{rep}_{s}", tag="negmax")
                nc.vector.tensor_reduce(negmax[:], logits_sb[:], axis=mybir.AxisListType.X,
                                        op=mybir.AluOpType.max, negate=True)
                ai = smallp.tile([1, T], f32, name=f"ai{rep}_{s}", tag="ai")
                sumexp = smallp.tile([1, 1], f32, name=f"se{rep}_{s}", tag="sumexp")
                nc.scalar.activation(ai[:], logits_sb[:], Exp, bias=negmax[0:1, 0:1],
                                     scale=1.0, accum_out=sumexp[0:1, 0:1])
                den = smallp.tile([1, 1], f32, name=f"den{rep}_{s}", tag="den")
                nc.vector.tensor_scalar_add(den[:], sumexp[:], EPS)
                recip = smallp.tile([1, 1], f32, name=f"rc{rep}_{s}", tag="recip")
                nc.vector.reciprocal(recip[:], den[:])
                att_sb = smallp.tile([4, T], f32r, name=f"att{rep}_{s}", tag="att")
                nc.vector.tensor_scalar_mul(att_sb[0:1, :], ai[:], recip[0:1, 0:1])
                if rep == 0:
                    nc.sync.dma_start(att_d[s:s + 1, :], att_sb[0:1, :].bitcast(f32))

                # ---- attT via 16 micro-transposes ----
                attT = smallp.tile([128, TC], f32r, name=f"attT{rep}_{s}", tag="attT")
                for j in range(TC):
                    tp = ps_tr.tile([128, 128], f32r, name=f"atp{rep}_{s}_{j}", tag="tr")
                    nc.tensor.matmul(tp[0:128, 0:4], att_sb[0:4, 128 * j:128 * (j + 1)],
                                     ident[0:4, 0:4], is_transpose=True)
                    nc.vector.tensor_copy(attT[:, j:j + 1], tp[0:128, 0:1])

                # ---- wsum: result[s] = sum_t att[t] * many[s,t,:] ----
                res = ps_lg.tile([1, 1024], f32, name=f"res{rep}_{s}", tag="lgres")
                for j in range(TC):
                    mw = natp.tile([128, D], f32r, name=f"mw{rep}_{s}_{j}", tag="mw")
                    nc.sync.dma_start(mw[:], many_d[s, 128 * j:128 * (j + 1), :])
                    for dh in range(2):
                        nc.tensor.matmul(res[0:1, 512 * dh:512 * (dh + 1)],
                                         attT[:, j:j + 1], mw[:, 512 * dh:512 * (dh + 1)],
                                         start=(j == 0), stop=(j == TC - 1))
                res_sb = smallp.tile([1, 1024], f32, name=f"ressb{rep}_{s}", tag="ressb")
                nc.vector.tensor_copy(res_sb[:], res[:])
                if rep == 0:
                    nc.sync.dma_start(res_d[s:s + 1, :], res_sb[:])

        if reps == 1:
            body(0)
        else:
            for rp in range(reps):
                body(rp)

    nc.compile()
    return nc


class _Runner:
    def __init__(self, nc, n_cores=NCORES):
        import jax
        import numpy as np
        from jax.sharding import Mesh, PartitionSpec
        from jax.experimental.shard_map import shard_map
        import concourse.mybir as mybir
        from concourse.bass2jax import (_bass_exec_p, partition_id_tensor,
                                        install_neuronx_cc_hook)
        install_neuronx_cc_hook()
        self.jax = jax
        self.n_cores = n_cores
        partition_name = nc.partition_id_tensor.name if nc.partition_id_tensor else None
        in_names, out_names, out_avals, zero_outs = [], [], [], []
        for alloc in nc.m.functions[0].allocations:
            if not isinstance(alloc, mybir.MemoryLocationSet):
                continue
            name = alloc.memorylocations[0].name
            if alloc.kind == "ExternalInput":
                if name != partition_name:
                    in_names.append(name)
            elif alloc.kind == "ExternalOutput":
                out_names.append(name)
                shape = tuple(alloc.tensor_shape)
                dtype = mybir.dt.np(alloc.dtype)
                out_avals.append(jax.core.ShapedArray(shape, dtype))
                zero_outs.append(np.zeros(shape, dtype))
        self.in_names, self.out_names = in_names, out_names
        self.out_avals, self.zero_outs = out_avals, zero_outs
        n_params, n_outs = len(in_names), len(out_avals)
        all_in = list(in_names) + list(out_names)
        if partition_name is not None:
            all_in.append(partition_name)
        donate = tuple(range(n_params, n_params + n_outs))

        def _body(*args):
            operands = list(args)
            if partition_name is not None:
                operands.append(partition_id_tensor())
            return tuple(_bass_exec_p.bind(
                *operands, out_avals=tuple(out_avals), in_names=tuple(all_in),
                out_names=tuple(out_names), lowering_input_output_aliases=(),
                sim_require_finite=True, sim_require_nnan=True, nc=nc))

        devices = jax.devices()[:n_cores]
        self.mesh = Mesh(np.asarray(devices), ("core",))
        in_specs = (PartitionSpec("core"),) * (n_params + n_outs)
        out_specs = (PartitionSpec("core"),) * len(out_names)
        self.fn = jax.jit(
            shard_map(_body, mesh=self.mesh, in_specs=in_specs,
                      out_specs=out_specs, check_rep=False),
            donate_argnums=donate, keep_unused=True)
        self.n_params = n_params
        self.PartitionSpec = PartitionSpec

    def put_inputs(self, in_maps):
        import numpy as np
        per_core = [[np.asarray(m[name]) for name in self.in_names] for m in in_maps]
        concat_in = [np.concatenate([per_core[c][i] for c in range(self.n_cores)], axis=0)
                     for i in range(self.n_params)]
        sharding = self.jax.sharding.NamedSharding(self.mesh, self.PartitionSpec("core"))
        return [self.jax.device_put(x, sharding) for x in concat_in]

    def run(self, dev_inputs):
        import numpy as np
        zeros = [np.zeros((self.n_cores * z.shape[0], *z.shape[1:]), z.dtype)
                 for z in self.zero_outs]
        out = self.fn(*dev_inputs, *zeros)
        self.jax.block_until_ready(out)
        return out

    def split_outputs(self, out_arrs):
        import numpy as np
        return {name: np.asarray(out_arrs[i]) for i, name in enumerate(self.out_names)}


def _get_runner(reps: int = 1):
    key = ("runner", reps)
    if key not in _CACHE:
        nc = _build(reps)
        _CACHE[key] = _Runner(nc)
    return _CACHE[key]


def kernel(one: np.ndarray, many: np.ndarray, W: np.ndarray):
    one = np.ascontiguousarray(one, dtype=np.float32)
    many = np.ascontiguousarray(many, dtype=np.float32)
    W = np.ascontiguousarray(W, dtype=np.float32)
    r = _get_runner(1)
    in_maps = [{"one": one[BLOC * c:BLOC * (c + 1)],
                "many": many[BLOC * c:BLOC * (c + 1)],
                "W": W} for c in range(NCORES)]
    dev_in = r.put_inputs(in_maps)
    out = r.run(dev_in)
    outs = r.split_outputs(out)
    result = outs["result"].reshape(B, D)
    att = outs["att"].reshape(B, T)
    return result, att


# revision 8
# speedup vs baseline: 1.9794x; 1.9794x over previous
"""Trainium2 Bass kernel for nn_ATT_ONE2MANY_1.

reference:
    proj = tanh(many @ W)                 # [B,T,D]
    logits = einsum('btd,bd->bt', proj, one)
    att = softmax(logits) with EPS=1e-7 in denominator
    result = einsum('btd,bt->bd', many, att)
    returns (result, att)

Sharding: data-parallel over batch B=32 across 8 cores (4 samples/core),
W replicated. No cross-core comms.

Per-core dataflow (matmuls in fp16 for full PE rate + fast weight load;
accumulation is always fp32 in PSUM; softmax in fp32):
  for s in 0..3:
    for h in 0..1 (T-halves):
      cast many tiles f32->f16 on GPSIMD, transpose on PE (fp16)
        -> manyT[e][128, 1024]  (8 e-chunks, kept for the whole sample)
      for d0 in 0..7:
        projT psum[128,512]x2 accumulated over e (W[e,d0] fp16 stationary)
        tanh (ACT, PSUM->SBUF, fp16)
        logits[1,512]x2 psum += oneT[:,d0,s].T @ tanhproj  (fp16, accum over d0)
    softmax on [1, 2048] row (exact max, exp on ACT with accum sum, EPS=1e-7)
    att broadcast to [128, T] fp16 via DMA
    wsum on DVE: tensor_tensor_reduce over manyT[e] x att_bcast -> resT columns,
    then one PE transpose -> result row
"""

import numpy as np

B, T, D = 32, 2048, 1024
NCORES = 8
BLOC = B // NCORES  # 4
EPS = 1e-7

_CACHE = {}


def _build(reps: int = 1):
    """Build the Bass program (one core's SPMD program). reps>1 replicates the
    whole body for wall-clock timing."""
    from contextlib import ExitStack
    import concourse.bass as bass
    import concourse.tile as tile
    import concourse.mybir as mybir
    from concourse import bacc
    from concourse.masks import make_identity

    f32 = mybir.dt.float32
    f16 = mybir.dt.float16
    Tanh = mybir.ActivationFunctionType.Tanh
    Exp = mybir.ActivationFunctionType.Exp

    nc = bacc.Bacc("TRN2", debug=False, num_devices=NCORES)

    one_d = nc.dram_tensor("one", [BLOC, D], f32, kind="ExternalInput").ap()
    many_d = nc.dram_tensor("many", [BLOC, T, D], f32, kind="ExternalInput").ap()
    w_d = nc.dram_tensor("W", [D, D], f32, kind="ExternalInput").ap()
    res_d = nc.dram_tensor("result", [BLOC, D], f32, kind="ExternalOutput").ap()
    att_d = nc.dram_tensor("att", [BLOC, T], f32, kind="ExternalOutput").ap()
    att_scr = nc.dram_tensor("att_scr", [BLOC, T], f32).ap()

    ED = D // 128   # 8 e-chunks (contraction dim of mm1)
    ND = D // 128   # 8 d0-chunks (output dim of mm1)
    TC = T // 128   # 16 t-chunks per sample
    HC = TC // 2    # 8 t-chunks per half

    with tile.TileContext(nc) as tc, ExitStack() as ctx:
        # ---- pools ----
        consts = ctx.enter_context(tc.tile_pool(name="consts", bufs=1))
        natp = ctx.enter_context(tc.tile_pool(name="natp", bufs=3))
        nat16p = ctx.enter_context(tc.tile_pool(name="nat16p", bufs=3))
        mtp = ctx.enter_context(tc.tile_pool(name="mtp", bufs=3))
        thp = ctx.enter_context(tc.tile_pool(name="thp", bufs=3))
        smallp = ctx.enter_context(tc.tile_pool(name="smallp", bufs=2))
        # PSUM banks: tr(2) + pj0(2) + pj1(2) + lgres(2 in one slot) = 8
        ps_tr = ctx.enter_context(tc.tile_pool(name="ps_tr", bufs=2, space="PSUM"))
        ps_pj = ctx.enter_context(tc.tile_pool(name="ps_pj", bufs=2, space="PSUM"))
        ps_lg = ctx.enter_context(tc.tile_pool(name="ps_lg", bufs=1, space="PSUM"))

        # ---- constants / setup ----
        ident_f = consts.tile([128, 128], f32, tag="identf")
        make_identity(nc, ident_f[:])
        ident = consts.tile([128, 128], f16, tag="ident")
        nc.vector.tensor_copy(ident[:], ident_f[:])

        w_sb = []
        for e in range(ED):
            wt32 = natp.tile([128, D], f32, name=f"w32_{e}", tag="nat")
            nc.sync.dma_start(wt32[:], w_d[128 * e:128 * (e + 1), :])
            wt = consts.tile([128, D], f16, name=f"w{e}", tag=f"w{e}")
            nc.vector.tensor_copy(wt[:], wt32[:])
            w_sb.append(wt)

        one32 = natp.tile([BLOC, D], f32, name="one32", tag="nat")
        nc.sync.dma_start(one32[:], one_d[:])
        one_sb = consts.tile([BLOC, D], f16, tag="one")
        nc.vector.tensor_copy(one_sb[:], one32[:])
        # oneT[:, d0*BLOC + s] = one[s, 128*d0 + :]
        oneT = consts.tile([128, ND * BLOC], f16, tag="oneT")
        for d0 in range(ND):
            tp = ps_tr.tile([128, 128], f16, name=f"onetp{d0}", tag="tr")
            nc.tensor.matmul(tp[0:128, 0:BLOC], one_sb[0:BLOC, 128 * d0:128 * (d0 + 1)],
                             ident[0:BLOC, 0:BLOC], is_transpose=True)
            nc.vector.tensor_copy(oneT[:, BLOC * d0:BLOC * (d0 + 1)], tp[0:128, 0:BLOC])

        import os
        _nsamp = int(os.environ.get("K_NSAMP", BLOC))
        _nowsum = os.environ.get("K_NOWSUM", "") == "1"

        def body(rep):
            for s in range(_nsamp):
                logits_sb = smallp.tile([1, T], f32, name=f"lgs{rep}_{s}", tag="logits")
                manyT_all = []
                for h in range(2):
                    # ---- cast + transpose: manyT[e] [128(e), 1024(t-half)] fp16 ----
                    manyT = [mtp.tile([128, 8 * 128], f16, name=f"mT{rep}_{s}_{h}_{e}", tag=f"mT{h}_{e}")
                             for e in range(ED)]
                    manyT_all.append(manyT)
                    for c in range(HC):
                        t0 = 1024 * h + 128 * c
                        nat = natp.tile([128, D], f32, name=f"nat{rep}_{s}_{h}_{c}", tag="nat")
                        nc.sync.dma_start(nat[:], many_d[s, t0:t0 + 128, :])
                        nat16 = nat16p.tile([128, D], f16, name=f"n16_{rep}_{s}_{h}_{c}", tag="nat16")
                        nc.gpsimd.tensor_copy(nat16[:], nat[:])
                        for e in range(ED):
                            tp = ps_tr.tile([128, 128], f16, name=f"tp{rep}_{s}_{h}_{c}_{e}", tag="tr")
                            nc.tensor.matmul(tp[:], nat16[:, 128 * e:128 * (e + 1)], ident[:],
                                             is_transpose=True)
                            if e % 2 == 0:
                                nc.vector.tensor_copy(manyT[e][:, 128 * c:128 * (c + 1)], tp[:])
                            else:
                                nc.scalar.copy(manyT[e][:, 128 * c:128 * (c + 1)], tp[:])
                    # ---- mm1 + tanh + mm2 ----
                    lg = ps_lg.tile([1, 1024], f32, name=f"lg{rep}_{s}_{h}", tag="lgres")
                    for d0 in range(ND):
                        pj = [ps_pj.tile([128, 512], f32, name=f"pj{rep}_{s}_{h}_{d0}_{tb}", tag=f"pj{tb}")
                              for tb in range(2)]
                        for e in range(ED):
                            for tb in range(2):
                                nc.tensor.matmul(
                                    pj[tb][:],
                                    w_sb[e][:, 128 * d0:128 * (d0 + 1)],
                                    manyT[e][:, 512 * tb:512 * (tb + 1)],
                                    start=(e == 0), stop=(e == ED - 1))
                        th = thp.tile([128, 1024], f16, name=f"th{rep}_{s}_{h}_{d0}", tag="th")
                        for tb in range(2):
                            nc.scalar.activation(th[:, 512 * tb:512 * (tb + 1)], pj[tb][:], Tanh)
                        ocol = oneT[:, BLOC * d0 + s:BLOC * d0 + s + 1]
                        for tb in range(2):
                            nc.tensor.matmul(lg[0:1, 512 * tb:512 * (tb + 1)],
                                             ocol, th[:, 512 * tb:512 * (tb + 1)],
                                             start=(d0 == 0), stop=(d0 == ND - 1))
                    nc.vector.tensor_copy(logits_sb[0:1, 1024 * h:1024 * (h + 1)], lg[:])

                # ---- softmax on [1, T] (partition 0), fp32 ----
                negmax = smallp.tile([1, 1], f32, name=f"nm{rep}_{s}", tag="negmax")
                nc.vector.tensor_reduce(negmax[:], logits_sb[:], axis=mybir.AxisListType.X,
                                        op=mybir.AluOpType.max, negate=True)
                ai = smallp.tile([1, T], f32, name=f"ai{rep}_{s}", tag="ai")
                sumexp = smallp.tile([1, 1], f32, name=f"se{rep}_{s}", tag="sumexp")
                nc.scalar.activation(ai[:], logits_sb[:], Exp, bias=negmax[0:1, 0:1],
                                     scale=1.0, accum_out=sumexp[0:1, 0:1])
                den = smallp.tile([1, 1], f32, name=f"den{rep}_{s}", tag="den")
                nc.vector.tensor_scalar_add(den[:], sumexp[:], EPS)
                recip = smallp.tile([1, 1], f32, name=f"rc{rep}_{s}", tag="recip")
                nc.vector.reciprocal(recip[:], den[:])
                att_sb = smallp.tile([1, T], f32, name=f"att{rep}_{s}", tag="att")
                nc.vector.tensor_scalar_mul(att_sb[:], ai[:], recip[0:1, 0:1])
                if rep == 0:
                    nc.sync.dma_start(att_d[s:s + 1, :], att_sb[0:1, :])
                nc.sync.dma_start(att_scr[s:s + 1, :], att_sb[0:1, :])

                if _nowsum:
                    continue
                # ---- wsum on DVE: resT[:, e] = sum_t manyT[e][:, t] * att[t] ----
                att_bc = smallp.tile([128, T], f16, name=f"attbc{rep}_{s}", tag="attbc")
                nc.gpsimd.dma_start(att_bc[:], att_scr[s, :].partition_broadcast(128))
                resT = smallp.tile([128, ED], f32, name=f"resT{rep}_{s}", tag="resT")
                for e in range(ED):
                    scratch = smallp.tile([128, T], f16, name=f"scr{rep}_{s}_{e}", tag="scr")
                    for h in range(2):
                        nc.vector.tensor_mul(scratch[:, 1024 * h:1024 * (h + 1)],
                                             manyT_all[h][e][:],
                                             att_bc[:, 1024 * h:1024 * (h + 1)])
                    nc.vector.tensor_reduce(resT[:, e:e + 1], scratch[:],
                                            axis=mybir.AxisListType.X,
                                            op=mybir.AluOpType.add)
                # resT [128, 8] -> result row via PE transpose ([8, 128] then dma)
                tpr = ps_tr.tile([128, 128], f32, name=f"tpr{rep}_{s}", tag="tr")
                nc.tensor.matmul(tpr[0:ED, 0:128], resT[0:128, 0:ED],
                                 ident_f[0:128, 0:128], is_transpose=True)
                res_sb = smallp.tile([ED, 128], f32, name=f"ressb{rep}_{s}", tag="ressb")
                nc.vector.tensor_copy(res_sb[:], tpr[0:ED, 0:128])
                if rep == 0:
                    nc.sync.dma_start(
                        res_d[s:s + 1, :].rearrange("o (e p) -> (o e) p", e=ED),
                        res_sb[:])

        for rp in range(reps):
            body(rp)

    nc.compile()
    return nc


class _Runner:
    def __init__(self, nc, n_cores=NCORES):
        import jax
        import numpy as np
        from jax.sharding import Mesh, PartitionSpec
        from jax.experimental.shard_map import shard_map
        import concourse.mybir as mybir
        from concourse.bass2jax import (_bass_exec_p, partition_id_tensor,
                                        install_neuronx_cc_hook)
        install_neuronx_cc_hook()
        self.jax = jax
        self.n_cores = n_cores
        partition_name = nc.partition_id_tensor.name if nc.partition_id_tensor else None
        in_names, out_names, out_avals, zero_outs = [], [], [], []
        for alloc in nc.m.functions[0].allocations:
            if not isinstance(alloc, mybir.MemoryLocationSet):
                continue
            name = alloc.memorylocations[0].name
            if alloc.kind == "ExternalInput":
                if name != partition_name:
                    in_names.append(name)
            elif alloc.kind == "ExternalOutput":
                out_names.append(name)
                shape = tuple(alloc.tensor_shape)
                dtype = mybir.dt.np(alloc.dtype)
                out_avals.append(jax.core.ShapedArray(shape, dtype))
                zero_outs.append(np.zeros(shape, dtype))
        self.in_names, self.out_names = in_names, out_names
        self.out_avals, self.zero_outs = out_avals, zero_outs
        n_params, n_outs = len(in_names), len(out_avals)
        all_in = list(in_names) + list(out_names)
        if partition_name is not None:
            all_in.append(partition_name)
        donate = tuple(range(n_params, n_params + n_outs))

        def _body(*args):
            operands = list(args)
            if partition_name is not None:
                operands.append(partition_id_tensor())
            return tuple(_bass_exec_p.bind(
                *operands, out_avals=tuple(out_avals), in_names=tuple(all_in),
                out_names=tuple(out_names), lowering_input_output_aliases=(),
                sim_require_finite=True, sim_require_nnan=True, nc=nc))

        devices = jax.devices()[:n_cores]
        self.mesh = Mesh(np.asarray(devices), ("core",))
        in_specs = (PartitionSpec("core"),) * (n_params + n_outs)
        out_specs = (PartitionSpec("core"),) * len(out_names)
        self.fn = jax.jit(
            shard_map(_body, mesh=self.mesh, in_specs=in_specs,
                      out_specs=out_specs, check_rep=False),
            donate_argnums=donate, keep_unused=True)
        self.n_params = n_params
        self.PartitionSpec = PartitionSpec

    def put_inputs(self, in_maps):
        import numpy as np
        per_core = [[np.asarray(m[name]) for name in self.in_names] for m in in_maps]
        concat_in = [np.concatenate([per_core[c][i] for c in range(self.n_cores)], axis=0)
                     for i in range(self.n_params)]
        sharding = self.jax.sharding.NamedSharding(self.mesh, self.PartitionSpec("core"))
        return [self.jax.device_put(x, sharding) for x in concat_in]

    def run(self, dev_inputs):
        import numpy as np
        zeros = [np.zeros((self.n_cores * z.shape[0], *z.shape[1:]), z.dtype)
                 for z in self.zero_outs]
        out = self.fn(*dev_inputs, *zeros)
        self.jax.block_until_ready(out)
        return out

    def split_outputs(self, out_arrs):
        import numpy as np
        return {name: np.asarray(out_arrs[i]) for i, name in enumerate(self.out_names)}


def _get_runner(reps: int = 1):
    key = ("runner", reps)
    if key not in _CACHE:
        nc = _build(reps)
        _CACHE[key] = _Runner(nc)
    return _CACHE[key]


def kernel(one: np.ndarray, many: np.ndarray, W: np.ndarray):
    one = np.ascontiguousarray(one, dtype=np.float32)
    many = np.ascontiguousarray(many, dtype=np.float32)
    W = np.ascontiguousarray(W, dtype=np.float32)
    r = _get_runner(1)
    in_maps = [{"one": one[BLOC * c:BLOC * (c + 1)],
                "many": many[BLOC * c:BLOC * (c + 1)],
                "W": W} for c in range(NCORES)]
    dev_in = r.put_inputs(in_maps)
    out = r.run(dev_in)
    outs = r.split_outputs(out)
    result = outs["result"].reshape(B, D)
    att = outs["att"].reshape(B, T)
    return result, att
